# revision 32
# baseline (speedup 1.0000x reference)
"""Trainium2 Bass kernel for nn_DiverseLoss (segment_reduce).

Math: with segment ids r = repeat(arange(N_SEG), BS) (aligned 8-row blocks),

    loss = 1 - sqrt( sum_e ||hs[e] - mean[e//BS]||^2 / E )

and per aligned block of BS rows:

    sum_j ||x_j - m||^2 = sum_j ||x_j||^2 - (1/BS) * ||sum_j x_j||^2

so the whole reduction is:  total_sumsq - total_blocksum_sq / BS.

Device strategy (8 cores, data-parallel over rows):
  * Each core streams its 32768x512 f32 shard in 32 "supertiles" of
    1024 rows, laid out [128 partitions x 4096 free] so partition p holds
    the 8 rows of one segment contiguously (16KB/partition contiguous DMA).
  * Block sums via TensorE: 8 accumulating matmuls with a 128x128 identity
    as the stationary operand sum the 8 rows of every segment into one
    PSUM bank [128, 512].
  * sum(x^2): split between VectorE (tensor_tensor_reduce, cols 0:1536) and
    ScalarE (activation Square with accum_out, cols 1536:4096).
  * sum(blocksum^2): VectorE tensor_tensor_reduce on the PSUM bank.
  * Per-supertile partials land in [128, 32] SBUF accumulator columns,
    DMA'd out once at the end; the final tiny reduction is float64 on host.
"""

import numpy as np

N_SEG = 32768
BS = 8
E = N_SEG * BS          # 262144 rows
D = 512
N_CORES = 8
R = E // N_CORES        # 32768 rows per core
P = 128
SUPER_ROWS = 1024       # rows per supertile
NS = R // SUPER_ROWS    # 32 supertiles per core
J = SUPER_ROWS // P     # 8 rows (one segment) per partition
FD = J * D              # 4096 f32 free elems per partition

_NC_CACHE = {}


def _build_nc(reps=1):
    """reps>1 repeats the whole pass in-kernel (used only for steady-state
    timing; accumulators are overwritten per pass so results are identical)."""
    import concourse.bacc as bacc
    import concourse.tile as tile
    from concourse import mybir
    from concourse.masks import make_identity

    f32 = mybir.dt.float32
    # Bacc (not plain Bass): its finalize() runs generate_event_semaphores,
    # which splits multi-semaphore waits into standalone event-semaphore
    # instructions — this walrus allows only ONE sync wait per instruction.
    nc = bacc.Bacc()
    hs_in = nc.declare_dram_parameter("hs_shard", [R, D], f32, isOutput=False)
    acc_out = nc.declare_dram_parameter("acc", [P, 2 * NS], f32, isOutput=True)

    hs_v = hs_in[:, :].rearrange("(s p j) d -> s p (j d)", p=P, j=J)

    with tile.TileContext(nc) as tc:
        with (
            tc.tile_pool(name="singles", bufs=1) as singles,
            tc.tile_pool(name="xpool", bufs=6) as xpool,
            tc.tile_pool(name="scratch", bufs=2) as scratch,
            tc.tile_pool(name="psum", bufs=8, space="PSUM") as psum,
        ):
            ident = singles.tile([P, P], f32)
            make_identity(nc, ident)
            acc_dve = singles.tile([P, NS], f32)
            acc_bs = singles.tile([P, NS], f32)

            for s in [s for _ in range(reps) for s in range(NS)]:
                x2 = xpool.tile([P, FD], f32)
                nc.sync.dma_start(out=x2, in_=hs_v[s])

                # block sums: 8 accumulating identity matmuls sum the 8 rows
                # of every segment into one PSUM bank [128, 512]
                ps = psum.tile([P, D], f32)
                for j in range(J):
                    nc.tensor.matmul(
                        ps,
                        ident,
                        x2[:, j * D : (j + 1) * D],
                        start=(j == 0),
                        stop=(j == J - 1),
                    )

                # sum of squares of raw rows — all on the VectorE, so x2 has
                # exactly two reader engines (PE + DVE). A third reader
                # engine costs ~15% steady-state time (extra wait
                # serialization on the x2 DMA issue path).
                sq_dve = scratch.tile([P, FD], f32)
                nc.vector.scalar_tensor_tensor(
                    out=sq_dve,
                    in0=x2,
                    scalar=1.0,
                    in1=x2,
                    op0=mybir.AluOpType.mult,
                    op1=mybir.AluOpType.mult,
                    accum_out=acc_dve[:, s : s + 1],
                )

                # sum of squared block sums: DVE copies the PSUM bank to SBUF
                # (one PSUM read port) then squares+accumulates. Keeping the
                # ScalarE away from PSUM matters: an ACT read of PSUM while
                # the PE streams into other banks costs ~30us/pass.
                bs_sb = scratch.tile([P, D], f32)
                nc.vector.tensor_copy(bs_sb, ps)
                sq_bs = scratch.tile([P, D], f32)
                nc.vector.scalar_tensor_tensor(
                    out=sq_bs,
                    in0=bs_sb,
                    scalar=1.0,
                    in1=bs_sb,
                    op0=mybir.AluOpType.mult,
                    op1=mybir.AluOpType.mult,
                    accum_out=acc_bs[:, s : s + 1],
                )

            nc.sync.dma_start(out=acc_out[:, 0:NS], in_=acc_dve)
            nc.sync.dma_start(out=acc_out[:, NS : 2 * NS], in_=acc_bs)

    # Runs Bacc's legalization pipeline (event-semaphore wait splitting,
    # ldweights wait hoisting, register allocation, ACT table loads).
    nc.finalize()
    return nc


def _get_nc():
    if "nc" not in _NC_CACHE:
        _NC_CACHE["nc"] = _build_nc()
    return _NC_CACHE["nc"]


def _run_device(hs, **kwargs):
    """hs: full [E, D] f32 array. Returns (per-core results, BassKernelResults)."""
    from concourse.bass_utils import run_bass_kernel_spmd

    nc = _get_nc()
    in_maps = [{"hs_shard": hs[c * R : (c + 1) * R]} for c in range(N_CORES)]
    res = run_bass_kernel_spmd(nc, in_maps, list(range(N_CORES)), **kwargs)
    return res


def _combine(results):
    total_sumsq = 0.0
    total_bs2 = 0.0
    for c in range(N_CORES):
        acc = np.asarray(results[c]["acc"], dtype=np.float64)
        total_sumsq += float(acc[:, :NS].sum())
        total_bs2 += float(acc[:, NS:].sum())
    total = total_sumsq - total_bs2 / BS
    return np.asarray(1.0 - np.sqrt(total / E), dtype=np.float32)


def _host_fallback(hs, bsv, edge_index):
    # General (unstructured segment ids) path; exact float64 reference math.
    r = np.asarray(edge_index)[:, 0].astype(np.int64)
    n_seg = hs.shape[0] // bsv
    hs64 = hs.astype(np.float64)
    seg_sum = np.zeros((n_seg, hs.shape[1]), dtype=np.float64)
    np.add.at(seg_sum, r, hs64)
    cnt = np.bincount(r, minlength=n_seg).astype(np.float64)
    mean = seg_sum / np.maximum(cnt, 1.0)[:, None]
    mean_t = np.repeat(mean, bsv, axis=0)
    total = ((hs64 - mean_t) ** 2).sum()
    return np.asarray(1.0 - np.sqrt(total / hs.shape[0]), dtype=np.float32)


def kernel(hs, bs, edge_index):
    hs = np.ascontiguousarray(np.asarray(hs), dtype=np.float32)
    bsv = int(np.asarray(bs))
    ei = np.asarray(edge_index)
    structured = (
        bsv == BS
        and hs.shape == (E, D)
        and np.array_equal(ei[:, 0], np.repeat(np.arange(N_SEG, dtype=ei.dtype), BS))
    )
    if not structured:
        return _host_fallback(hs, bsv, ei)
    res = _run_device(hs)
    return _combine(res.results)
